# revision 5
# baseline (speedup 1.0000x reference)
"""nn_DenseGeneral: AQT-style int8 fake-quant einsum 'btd,dh->bth' on 8 NeuronCores.

Math: fake-quant values are integers in [-127,127] -> exact in bf16; the
integer products accumulate exactly in fp32 PSUM. So:
    out = (sum_d qi[t,d]*qk[d,h]) * si[t] * sk[h]
with qi/qk computed as round(x * (1/s)) via the fp32 magic-number trick.

Two SPMD launches over 8 cores:
  A (quantize):  core c quantizes input rows [1024c:1024(c+1)] (scales are
     per-row, rows are complete) and kernel column-slice [:, 512c:512(c+1)]
     (scales are per-column, columns are complete). Outputs bf16 quantized
     tensors + fp32 scales.
  B (matmul):    row-parallel. Core c consumes its own quantized input rows
     (loaded D-major via DMA-xbar transpose), the full quantized kernel
     (host-concatenated between launches), computes out rows [1024, 4096]:
     bf16 matmuls accumulating in PSUM, fused epilogue (psum*si)*sk on DVE.
"""
import sys

if "/opt/trn_rl_repo" not in sys.path:
    sys.path.insert(0, "/opt/trn_rl_repo")

import numpy as np
import ml_dtypes

import concourse.bacc as bacc
import concourse.mybir as mybir
import concourse.tile as tile
from concourse import bass_isa
from concourse.bass2jax import (
    _bass_exec_p,
    install_neuronx_cc_hook,
    partition_id_tensor,
)

f32 = mybir.dt.float32
bf16 = mybir.dt.bfloat16
A_ = mybir.AluOpType
AX = mybir.AxisListType
AF = mybir.ActivationFunctionType

MAGIC = float(np.float32(1.5 * 2**23))   # fp32 round-to-int magic
C127 = float(np.float32(1.0 / 127.0))
EPS = 1e-8

NCORES = 8
B, T, D, H = 4, 2048, 1024, 4096
BT = B * T                 # 8192 rows total
TR = BT // NCORES          # 1024 rows per core
HS = H // NCORES           # 512 kernel cols per core
DCH = D // 128             # 8 contraction chunks
TT = TR // 128             # 8 T-tiles per core
NHALF = 2                  # output halves (PSUM double-buffer)
HH = H // NHALF            # 2048
NQ = HH // 512             # 4 matmuls of N=512 per half


def _build_prog_a2():
    """Launch A: quantize input rows + kernel column-slice."""
    nc = bacc.Bacc("TRN2", target_bir_lowering=False, debug=False)
    x_dram = nc.dram_tensor("xa", [TR, D], f32, kind="ExternalInput")
    k_dram = nc.dram_tensor("ka", [D, HS], f32, kind="ExternalInput")
    qnat = nc.dram_tensor("qnat", [TR, D], bf16, kind="ExternalOutput")
    si_o = nc.dram_tensor("si", [128, TT], f32, kind="ExternalOutput")
    qk_o = nc.dram_tensor("qk", [D, HS], bf16, kind="ExternalOutput")
    sk_o = nc.dram_tensor("sk", [1, HS], f32, kind="ExternalOutput")

    with tile.TileContext(nc) as tc:
        with (
            tc.tile_pool(name="sb", bufs=3) as sb,
            tc.tile_pool(name="kp", bufs=1) as kp,
            tc.tile_pool(name="sip", bufs=1) as sip,
        ):
            # ---------- kernel slice quantize ----------
            k_sb = kp.tile([128, DCH, HS], f32)
            for c in range(DCH):
                nc.sync.dma_start(k_sb[:, c, :], k_dram[c * 128:(c + 1) * 128, :])
            kmax = kp.tile([128, HS], f32)
            nc.vector.tensor_reduce(kmax[:], k_sb[:].rearrange("p c h -> p h c"),
                                    axis=AX.X, op=A_.max, apply_absolute_value=True)
            colmax = kp.tile([128, HS], f32)
            nc.gpsimd.partition_all_reduce(colmax[:], kmax[:], channels=128,
                                           reduce_op=bass_isa.ReduceOp.max)
            S_k = kp.tile([128, HS], f32)
            nc.vector.tensor_scalar(out=S_k[:], in0=colmax[:], scalar1=C127,
                                    scalar2=float(EPS), op0=A_.mult, op1=A_.max)
            nc.sync.dma_start(sk_o[:], S_k[0:1, :])
            R_k = kp.tile([128, HS], f32)
            nc.vector.reciprocal(R_k[:], S_k[:])
            qk_sb = kp.tile([128, DCH, HS], bf16)
            for c in range(DCH):
                tk = sb.tile([128, HS], f32, tag="tk")
                nc.vector.tensor_tensor(out=tk[:], in0=k_sb[:, c, :], in1=R_k[:],
                                        op=A_.mult)
                nc.vector.tensor_scalar(out=qk_sb[:, c, :], in0=tk[:],
                                        scalar1=MAGIC, scalar2=MAGIC,
                                        op0=A_.add, op1=A_.subtract)
                nc.sync.dma_start(qk_o[c * 128:(c + 1) * 128, :], qk_sb[:, c, :])

            # ---------- input rows quantize ----------
            si_sb = sip.tile([128, TT], f32)
            for t in range(TT):
                x_sb = sb.tile([128, D], f32, tag="x")
                nc.sync.dma_start(x_sb[:], x_dram[t * 128:(t + 1) * 128, :])
                rmax = sb.tile([128, 1], f32, tag="rmax")
                nc.vector.tensor_reduce(rmax[:], x_sb[:], axis=AX.X, op=A_.max,
                                        apply_absolute_value=True)
                nc.vector.tensor_scalar(out=si_sb[:, t:t + 1], in0=rmax[:],
                                        scalar1=C127, scalar2=float(EPS),
                                        op0=A_.mult, op1=A_.max)
                r_row = sb.tile([128, 1], f32, tag="rrow")
                nc.vector.reciprocal(r_row[:], si_sb[:, t:t + 1])
                t_sb = sb.tile([128, D], f32, tag="t")
                nc.vector.tensor_scalar(out=t_sb[:], in0=x_sb[:], scalar1=r_row[:],
                                        scalar2=MAGIC, op0=A_.mult, op1=A_.add)
                q_sb = sb.tile([128, D], bf16, tag="q")
                nc.scalar.activation(q_sb[:], t_sb[:], AF.Copy,
                                     bias=-MAGIC, scale=1.0)
                nc.sync.dma_start(qnat[t * 128:(t + 1) * 128, :], q_sb[:])
            nc.sync.dma_start(si_o[:], si_sb[:])
    nc.compile()
    return nc


def _build_prog_b():
    """Launch B: row-parallel bf16 matmul + fused scaling epilogue."""
    nc = bacc.Bacc("TRN2", target_bir_lowering=False, debug=False)
    qnat = nc.dram_tensor("qnat", [TR, D], bf16, kind="ExternalInput")
    si_i = nc.dram_tensor("si", [128, TT], f32, kind="ExternalInput")
    qk_i = nc.dram_tensor("qkf", [D, H], bf16, kind="ExternalInput")
    sk_i = nc.dram_tensor("skf", [1, H], f32, kind="ExternalInput")
    out_o = nc.dram_tensor("out", [TR, H], f32, kind="ExternalOutput")

    with tile.TileContext(nc) as tc:
        with (
            tc.tile_pool(name="wp", bufs=1) as wp,
            tc.tile_pool(name="ob", bufs=3) as ob,
            tc.tile_pool(name="pp", bufs=2, space="PSUM") as pp,
        ):
            # resident: full quantized kernel, transposed inputs, scales
            qk_sb = wp.tile([128, DCH, H], bf16)
            nc.sync.dma_start(
                qk_sb[:],
                qk_i[:].rearrange("(c p) h -> p c h", p=128))
            qiT = wp.tile([128, DCH, TR], bf16)
            for c in range(DCH):
                nc.sync.dma_start(qiT[:, c, :],
                                  qnat[:, c * 128:(c + 1) * 128], transpose=True)
            si_sb = wp.tile([128, TT], f32)
            nc.sync.dma_start(si_sb[:], si_i[:])
            sk_row = wp.tile([1, H], f32)
            nc.sync.dma_start(sk_row[:], sk_i[:])
            sk_b = wp.tile([128, H], f32)
            nc.gpsimd.partition_broadcast(sk_b[:], sk_row[:])

            for t in range(TT):
                for hh in range(NHALF):
                    ps = pp.tile([128, HH], f32, tag="ps")
                    for c in range(DCH):
                        lhsT = qiT[:, c, t * 128:(t + 1) * 128]
                        for q in range(NQ):
                            off = hh * HH + q * 512
                            nc.tensor.matmul(
                                ps[:, q * 512:(q + 1) * 512],
                                lhsT,
                                qk_sb[:, c, off:off + 512],
                                start=(c == 0), stop=(c == DCH - 1))
                    o_sb = ob.tile([128, HH], f32, tag="o")
                    nc.vector.scalar_tensor_tensor(
                        out=o_sb[:], in0=ps[:], scalar=si_sb[:, t:t + 1],
                        in1=sk_b[:, hh * HH:(hh + 1) * HH],
                        op0=A_.mult, op1=A_.mult)
                    nc.sync.dma_start(
                        out_o[t * 128:(t + 1) * 128, hh * HH:(hh + 1) * HH],
                        o_sb[:])
    nc.compile()
    return nc


# ---------------------------------------------------------------------------
# Runner: replicate bass2jax.run_bass_via_pjrt but cache the jitted callable.
# ---------------------------------------------------------------------------
class _Prog:
    def __init__(self, nc, n_cores=NCORES):
        import jax
        from jax.sharding import Mesh, PartitionSpec
        try:
            from jax.experimental.shard_map import shard_map
        except ImportError:
            from jax.shard_map import shard_map

        install_neuronx_cc_hook()
        self.nc = nc
        self.n_cores = n_cores
        partition_name = (nc.partition_id_tensor.name
                          if nc.partition_id_tensor else None)
        in_names, out_names, out_avals, zero_shapes = [], [], [], []
        for alloc in nc.m.functions[0].allocations:
            if not isinstance(alloc, mybir.MemoryLocationSet):
                continue
            name = alloc.memorylocations[0].name
            if alloc.kind == "ExternalInput":
                if name == partition_name:
                    continue
                in_names.append(name)
            elif alloc.kind == "ExternalOutput":
                out_names.append(name)
                shape = tuple(alloc.tensor_shape)
                dtype = mybir.dt.np(alloc.dtype)
                out_avals.append(jax.core.ShapedArray(shape, dtype))
                zero_shapes.append((shape, dtype))
        self.in_names = list(in_names)
        self.out_names = out_names
        self.out_avals = out_avals
        self.zero_shapes = zero_shapes
        n_params = len(in_names)
        n_outs = len(out_names)
        all_names = in_names + out_names
        if partition_name is not None:
            all_names = all_names + [partition_name]

        def _body(*args):
            operands = list(args)
            if partition_name is not None:
                operands.append(partition_id_tensor())
            outs = _bass_exec_p.bind(
                *operands,
                out_avals=tuple(out_avals),
                in_names=tuple(all_names),
                out_names=tuple(out_names),
                lowering_input_output_aliases=(),
                sim_require_finite=True,
                sim_require_nnan=True,
                nc=nc,
            )
            return tuple(outs)

        donate = tuple(range(n_params, n_params + n_outs))
        devices = jax.devices()[:n_cores]
        mesh = Mesh(np.asarray(devices), ("core",))
        in_specs = (PartitionSpec("core"),) * (n_params + n_outs)
        out_specs = (PartitionSpec("core"),) * n_outs
        self.fn = jax.jit(
            shard_map(_body, mesh=mesh, in_specs=in_specs,
                      out_specs=out_specs, check_rep=False),
            donate_argnums=donate, keep_unused=True)

    def concat_inputs(self, in_maps):
        return [
            np.concatenate([np.asarray(m[name]) for m in in_maps], axis=0)
            for name in self.in_names
        ]

    def fresh_zeros(self):
        return [np.zeros((self.n_cores * s[0], *s[1:]), d)
                for (s, d) in self.zero_shapes]

    def run(self, concat_in):
        out_arrs = self.fn(*concat_in, *self.fresh_zeros())
        return out_arrs

    def split(self, out_arrs):
        res = []
        for c in range(self.n_cores):
            res.append({
                name: np.asarray(out_arrs[i]).reshape(
                    self.n_cores, *self.out_avals[i].shape)[c]
                for i, name in enumerate(self.out_names)
            })
        return res


_progs = {}


def _get_progs():
    if "a" not in _progs:
        _progs["a"] = _Prog(_build_prog_a2())
        _progs["b"] = _Prog(_build_prog_b())
    return _progs["a"], _progs["b"]


def kernel(inputs: np.ndarray, kernel: np.ndarray) -> np.ndarray:
    pa, pb = _get_progs()
    x = np.ascontiguousarray(np.asarray(inputs, dtype=np.float32).reshape(BT, D))
    w = np.ascontiguousarray(np.asarray(kernel, dtype=np.float32))

    in_maps_a = [
        {"xa": x[c * TR:(c + 1) * TR], "ka": w[:, c * HS:(c + 1) * HS]}
        for c in range(NCORES)
    ]
    res_a = pa.split(pa.run(pa.concat_inputs(in_maps_a)))

    qk_full = np.concatenate([r["qk"] for r in res_a], axis=1)      # [D, H] bf16
    sk_full = np.concatenate([r["sk"] for r in res_a], axis=1)      # [1, H] f32

    in_maps_b = [
        {"qnat": res_a[c]["qnat"], "si": res_a[c]["si"],
         "qkf": qk_full, "skf": sk_full}
        for c in range(NCORES)
    ]
    res_b = pb.split(pb.run(pb.concat_inputs(in_maps_b)))

    out = np.concatenate([r["out"] for r in res_b], axis=0)         # [BT, H]
    return out.reshape(B, T, H)


# revision 6
# speedup vs baseline: 1772.5898x; 1772.5898x over previous
"""nn_DenseGeneral: AQT-style int8 fake-quant einsum 'btd,dh->bth' on 8 NeuronCores.

Math: fake-quant values are integers in [-127,127] -> exact in bf16; the
integer products accumulate exactly in fp32 PSUM. So:
    out = (sum_d qi[t,d]*qk[d,h]) * si[t] * sk[h]
with qi/qk computed as round(x * (1/s)) via the fp32 magic-number trick.

Two SPMD launches over 8 cores:
  A (quantize):  core c quantizes input rows [1024c:1024(c+1)] (scales are
     per-row, rows are complete) and kernel column-slice [:, 512c:512(c+1)]
     (scales are per-column, columns are complete). Outputs bf16 quantized
     tensors + fp32 scales.
  B (matmul):    row-parallel. Core c consumes its own quantized input rows
     (loaded D-major via DMA-xbar transpose), the full quantized kernel
     (host-concatenated between launches), computes out rows [1024, 4096]:
     bf16 matmuls accumulating in PSUM, fused epilogue (psum*si)*sk on DVE.
"""
import sys

if "/opt/trn_rl_repo" not in sys.path:
    sys.path.insert(0, "/opt/trn_rl_repo")

import numpy as np
import ml_dtypes

import concourse.bacc as bacc
import concourse.mybir as mybir
import concourse.tile as tile
from concourse import bass_isa
from concourse.bass2jax import (
    _bass_exec_p,
    install_neuronx_cc_hook,
    partition_id_tensor,
)

f32 = mybir.dt.float32
bf16 = mybir.dt.bfloat16
A_ = mybir.AluOpType
AX = mybir.AxisListType
AF = mybir.ActivationFunctionType

MAGIC = float(np.float32(1.5 * 2**23))   # fp32 round-to-int magic
C127 = float(np.float32(1.0 / 127.0))
EPS = 1e-8

NCORES = 8
B, T, D, H = 4, 2048, 1024, 4096
BT = B * T                 # 8192 rows total
TR = BT // NCORES          # 1024 rows per core
HS = H // NCORES           # 512 kernel cols per core
DCH = D // 128             # 8 contraction chunks
TT = TR // 128             # 8 T-tiles per core
NHALF = 2                  # output halves (PSUM double-buffer)
HH = H // NHALF            # 2048
NQ = HH // 512             # 4 matmuls of N=512 per half


def _build_prog_a2():
    """Launch A: quantize input rows + kernel column-slice."""
    nc = bacc.Bacc("TRN2", target_bir_lowering=False, debug=False)
    x_dram = nc.dram_tensor("xa", [TR, D], f32, kind="ExternalInput")
    k_dram = nc.dram_tensor("ka", [D, HS], f32, kind="ExternalInput")
    qnat = nc.dram_tensor("qnat", [TR, D], bf16, kind="ExternalOutput")
    si_o = nc.dram_tensor("si", [128, TT], f32, kind="ExternalOutput")
    qk_o = nc.dram_tensor("qk", [D, HS], bf16, kind="ExternalOutput")
    sk_o = nc.dram_tensor("sk", [1, HS], f32, kind="ExternalOutput")

    with tile.TileContext(nc) as tc:
        with (
            tc.tile_pool(name="sb", bufs=3) as sb,
            tc.tile_pool(name="kp", bufs=1) as kp,
            tc.tile_pool(name="sip", bufs=1) as sip,
        ):
            # ---------- kernel slice quantize ----------
            k_sb = kp.tile([128, DCH, HS], f32)
            for c in range(DCH):
                nc.sync.dma_start(k_sb[:, c, :], k_dram[c * 128:(c + 1) * 128, :])
            kmax = kp.tile([128, HS], f32)
            nc.vector.tensor_reduce(kmax[:], k_sb[:].rearrange("p c h -> p h c"),
                                    axis=AX.X, op=A_.max, apply_absolute_value=True)
            colmax = kp.tile([128, HS], f32)
            nc.gpsimd.partition_all_reduce(colmax[:], kmax[:], channels=128,
                                           reduce_op=bass_isa.ReduceOp.max)
            S_k = kp.tile([128, HS], f32)
            nc.vector.tensor_scalar(out=S_k[:], in0=colmax[:], scalar1=C127,
                                    scalar2=float(EPS), op0=A_.mult, op1=A_.max)
            nc.sync.dma_start(sk_o[:], S_k[0:1, :])
            R_k = kp.tile([128, HS], f32)
            nc.vector.reciprocal(R_k[:], S_k[:])
            qk_sb = kp.tile([128, DCH, HS], bf16)
            for c in range(DCH):
                tk = sb.tile([128, HS], f32, tag="tk")
                nc.vector.tensor_tensor(out=tk[:], in0=k_sb[:, c, :], in1=R_k[:],
                                        op=A_.mult)
                nc.vector.tensor_scalar(out=qk_sb[:, c, :], in0=tk[:],
                                        scalar1=MAGIC, scalar2=MAGIC,
                                        op0=A_.add, op1=A_.subtract)
                nc.sync.dma_start(qk_o[c * 128:(c + 1) * 128, :], qk_sb[:, c, :])

            # ---------- input rows quantize ----------
            si_sb = sip.tile([128, TT], f32)
            for t in range(TT):
                x_sb = sb.tile([128, D], f32, tag="x")
                nc.sync.dma_start(x_sb[:], x_dram[t * 128:(t + 1) * 128, :])
                rmax = sb.tile([128, 1], f32, tag="rmax")
                nc.vector.tensor_reduce(rmax[:], x_sb[:], axis=AX.X, op=A_.max,
                                        apply_absolute_value=True)
                nc.vector.tensor_scalar(out=si_sb[:, t:t + 1], in0=rmax[:],
                                        scalar1=C127, scalar2=float(EPS),
                                        op0=A_.mult, op1=A_.max)
                r_row = sb.tile([128, 1], f32, tag="rrow")
                nc.vector.reciprocal(r_row[:], si_sb[:, t:t + 1])
                t_sb = sb.tile([128, D], f32, tag="t")
                nc.vector.tensor_scalar(out=t_sb[:], in0=x_sb[:], scalar1=r_row[:],
                                        scalar2=MAGIC, op0=A_.mult, op1=A_.add)
                q_sb = sb.tile([128, D], bf16, tag="q")
                nc.scalar.activation(q_sb[:], t_sb[:], AF.Copy,
                                     bias=-MAGIC, scale=1.0)
                nc.sync.dma_start(qnat[t * 128:(t + 1) * 128, :], q_sb[:])
            nc.sync.dma_start(si_o[:], si_sb[:])
    nc.compile()
    return nc


def _build_prog_b():
    """Launch B: row-parallel bf16 matmul + fused scaling epilogue."""
    nc = bacc.Bacc("TRN2", target_bir_lowering=False, debug=False)
    qnat = nc.dram_tensor("qnat", [TR, D], bf16, kind="ExternalInput")
    si_i = nc.dram_tensor("si", [128, TT], f32, kind="ExternalInput")
    qk_i = nc.dram_tensor("qkf", [D, H], bf16, kind="ExternalInput")
    sk_i = nc.dram_tensor("skf", [1, H], f32, kind="ExternalInput")
    out_o = nc.dram_tensor("out", [TR, H], f32, kind="ExternalOutput")

    with tile.TileContext(nc) as tc:
        with (
            tc.tile_pool(name="wp", bufs=1) as wp,
            tc.tile_pool(name="ob", bufs=3) as ob,
            tc.tile_pool(name="pp", bufs=2, space="PSUM") as pp,
        ):
            # resident: full quantized kernel, transposed inputs, scales
            qk_sb = wp.tile([128, DCH, H], bf16)
            nc.sync.dma_start(
                qk_sb[:],
                qk_i[:].rearrange("(c p) h -> p c h", p=128))
            qiT = wp.tile([128, DCH, TR], bf16)
            for c in range(DCH):
                nc.sync.dma_start(qiT[:, c, :],
                                  qnat[:, c * 128:(c + 1) * 128], transpose=True)
            si_sb = wp.tile([128, TT], f32)
            nc.sync.dma_start(si_sb[:], si_i[:])
            sk_row = wp.tile([1, H], f32)
            nc.sync.dma_start(sk_row[:], sk_i[:])
            sk_b = wp.tile([128, H], f32)
            nc.gpsimd.partition_broadcast(sk_b[:], sk_row[:])

            for t in range(TT):
                for hh in range(NHALF):
                    ps = pp.tile([128, HH], f32, tag="ps")
                    for c in range(DCH):
                        lhsT = qiT[:, c, t * 128:(t + 1) * 128]
                        for q in range(NQ):
                            off = hh * HH + q * 512
                            nc.tensor.matmul(
                                ps[:, q * 512:(q + 1) * 512],
                                lhsT,
                                qk_sb[:, c, off:off + 512],
                                start=(c == 0), stop=(c == DCH - 1))
                    o_sb = ob.tile([128, HH], f32, tag="o")
                    nc.vector.scalar_tensor_tensor(
                        out=o_sb[:], in0=ps[:], scalar=si_sb[:, t:t + 1],
                        in1=sk_b[:, hh * HH:(hh + 1) * HH],
                        op0=A_.mult, op1=A_.mult)
                    nc.sync.dma_start(
                        out_o[t * 128:(t + 1) * 128, hh * HH:(hh + 1) * HH],
                        o_sb[:])
    nc.compile()
    return nc


# ---------------------------------------------------------------------------
# Runner: replicate bass2jax.run_bass_via_pjrt but cache the jitted callable.
# ---------------------------------------------------------------------------
class _Prog:
    def __init__(self, nc, n_cores=NCORES):
        import jax
        from jax.sharding import Mesh, PartitionSpec
        try:
            from jax.experimental.shard_map import shard_map
        except ImportError:
            from jax.shard_map import shard_map

        install_neuronx_cc_hook()
        self.nc = nc
        self.n_cores = n_cores
        partition_name = (nc.partition_id_tensor.name
                          if nc.partition_id_tensor else None)
        in_names, out_names, out_avals, zero_shapes = [], [], [], []
        for alloc in nc.m.functions[0].allocations:
            if not isinstance(alloc, mybir.MemoryLocationSet):
                continue
            name = alloc.memorylocations[0].name
            if alloc.kind == "ExternalInput":
                if name == partition_name:
                    continue
                in_names.append(name)
            elif alloc.kind == "ExternalOutput":
                out_names.append(name)
                shape = tuple(alloc.tensor_shape)
                dtype = mybir.dt.np(alloc.dtype)
                out_avals.append(jax.core.ShapedArray(shape, dtype))
                zero_shapes.append((shape, dtype))
        self.in_names = list(in_names)
        self.out_names = out_names
        self.out_avals = out_avals
        self.zero_shapes = zero_shapes
        n_params = len(in_names)
        n_outs = len(out_names)
        all_names = in_names + out_names
        if partition_name is not None:
            all_names = all_names + [partition_name]

        def _body(*args):
            operands = list(args)
            if partition_name is not None:
                operands.append(partition_id_tensor())
            outs = _bass_exec_p.bind(
                *operands,
                out_avals=tuple(out_avals),
                in_names=tuple(all_names),
                out_names=tuple(out_names),
                lowering_input_output_aliases=(),
                sim_require_finite=True,
                sim_require_nnan=True,
                nc=nc,
            )
            return tuple(outs)

        donate = tuple(range(n_params, n_params + n_outs))
        devices = jax.devices()[:n_cores]
        mesh = Mesh(np.asarray(devices), ("core",))
        self.mesh = mesh
        self.PartitionSpec = PartitionSpec
        self.n_params = n_params
        self.n_outs = n_outs
        in_specs = (PartitionSpec("core"),) * (n_params + n_outs)
        out_specs = (PartitionSpec("core"),) * n_outs
        self._body = _body
        self._shard_map = shard_map
        self.fn = jax.jit(
            shard_map(_body, mesh=mesh, in_specs=in_specs,
                      out_specs=out_specs, check_rep=False),
            donate_argnums=donate, keep_unused=True)
        self._chained = {}

    def chained_fn(self, n):
        """jit fn executing the NEFF n times sequentially (for timing)."""
        import jax

        if n in self._chained:
            return self._chained[n]

        def _body_n(*args):
            outs = None
            for _ in range(n):
                outs = self._body(*args)
            return outs

        in_specs = (self.PartitionSpec("core"),) * (self.n_params + self.n_outs)
        out_specs = (self.PartitionSpec("core"),) * self.n_outs
        fn = jax.jit(
            self._shard_map(_body_n, mesh=self.mesh, in_specs=in_specs,
                            out_specs=out_specs, check_rep=False),
            keep_unused=True)
        self._chained[n] = fn
        return fn

    def device_inputs(self, concat_in):
        """device_put inputs with the mesh sharding (axis 0 split)."""
        import jax
        from jax.sharding import NamedSharding

        sharding = NamedSharding(self.mesh, self.PartitionSpec("core"))
        out = [jax.device_put(a, sharding) for a in concat_in]
        for a in out:
            a.block_until_ready()
        return out

    def concat_inputs(self, in_maps):
        return [
            np.concatenate([np.asarray(m[name]) for m in in_maps], axis=0)
            for name in self.in_names
        ]

    def fresh_zeros(self):
        return [np.zeros((self.n_cores * s[0], *s[1:]), d)
                for (s, d) in self.zero_shapes]

    def run(self, concat_in):
        out_arrs = self.fn(*concat_in, *self.fresh_zeros())
        return out_arrs

    def split(self, out_arrs):
        res = []
        for c in range(self.n_cores):
            res.append({
                name: np.asarray(out_arrs[i]).reshape(
                    self.n_cores, *self.out_avals[i].shape)[c]
                for i, name in enumerate(self.out_names)
            })
        return res


_progs = {}


def _get_progs():
    if "a" not in _progs:
        _progs["a"] = _Prog(_build_prog_a2())
        _progs["b"] = _Prog(_build_prog_b())
    return _progs["a"], _progs["b"]


def kernel(inputs: np.ndarray, kernel: np.ndarray) -> np.ndarray:
    pa, pb = _get_progs()
    x = np.ascontiguousarray(np.asarray(inputs, dtype=np.float32).reshape(BT, D))
    w = np.ascontiguousarray(np.asarray(kernel, dtype=np.float32))

    in_maps_a = [
        {"xa": x[c * TR:(c + 1) * TR], "ka": w[:, c * HS:(c + 1) * HS]}
        for c in range(NCORES)
    ]
    res_a = pa.split(pa.run(pa.concat_inputs(in_maps_a)))

    qk_full = np.concatenate([r["qk"] for r in res_a], axis=1)      # [D, H] bf16
    sk_full = np.concatenate([r["sk"] for r in res_a], axis=1)      # [1, H] f32

    in_maps_b = [
        {"qnat": res_a[c]["qnat"], "si": res_a[c]["si"],
         "qkf": qk_full, "skf": sk_full}
        for c in range(NCORES)
    ]
    res_b = pb.split(pb.run(pb.concat_inputs(in_maps_b)))

    out = np.concatenate([r["out"] for r in res_b], axis=0)         # [BT, H]
    return out.reshape(B, T, H)
